# revision 23
# baseline (speedup 1.0000x reference)
"""Conv2d 3x3 (stride 1, pad 1) as implicit GEMM on 8 Trainium2 NeuronCores.

Problem: x[32,128,56,56] f32, weights[128,128,3,3] f32, bias[128] f32
         -> out[32,128,56,56] f32.

Sharding: data-parallel over batch — 4 images per core, weights/bias
replicated on every core.

Per-core kernel design (v4b):
  - channels (128) live on the SBUF partition dim.
  - the host pre-pads each image into a flat row layout
      [margin 58 | 56 rows x (56 data + 1 zero) | margin 58]
    so every conv tap (kh,kw) is a plain shifted window d=(kh-1)*57+(kw-1)
    of one flat buffer, and the device DMA is fully contiguous.
  - taps 0-6 run in bf16 (1 col/cycle floor, ~193ns per 456-col matmul);
    taps 7+8 are packed into ONE fp8e4m3 DoubleRow matmul (2 taps per
    pass, ~1.13x a bf16 matmul) -> each 9-tap group costs ~7.13 matmul
    slots instead of 9.  Host supplies x both as bf16 flat tiles and as
    an fp8 two-plane tile (plane1 = plane0 shifted by one column) so the
    DR rhs is a clean 3D AP [128][2][N]; measured end-to-end rel err
    1.66e-2 < 2e-2 gate (deterministic fixed-seed inputs).
  - PSUM accumulates fp32 for all taps; scalar evicts PSUM->SBUF with
    fused bias (dropping pad columns), output leaves as bf16, host
    upcasts.
  - head: per-DMA-queue bandwidth is only ~140 GB/s and DGE startup is
    ~1.3us, so the critical transfers are paced: sync carries weights in
    tap order (chunks sized to the *cold* 380ns tap pitch), scalar
    carries x0's bf16 chunk0; the fp8 planes arrive later (pair is the
    LAST matmul of each group).  PE warm-up (HAM 1.2->2.4GHz un-throttle
    needs ~3.4us of sustained activity) uses a small memset + 13 narrow
    matmuls bridging user-code entry (~7.5us) to data-ready (~10us).
  - tail: last image ends with 6-row and 2-row groups; the 6-row output
    DMA is triggered by scalar (after its own evict), the final 2-row
    evict runs on DVE and sync triggers its DMA; DGE completion latency
    (~1.4us) bounds the tail.
"""

import numpy as np

N_TOTAL = 32
N_CORES = 8
N_PER_CORE = N_TOTAL // N_CORES
C = 128
H = W = 56
HW = H * W            # 3136
WP = W + 1            # 57  padded row width (shared pad col)
L = H * WP            # 3192 flat padded length
MARGIN = WP + 1       # 58  covers worst tap offset
TILE_W = MARGIN + L + MARGIN  # 3308
GW = 8 * WP           # 456 (<=512 fp32 PSUM bank)
N_WARM = 12
WARM_N = 228
# image-0 bf16 chunk bounds after the head chunk [0:572)
X_BOUNDS0 = [0, MARGIN + GW + MARGIN, MARGIN + 3 * GW + MARGIN,
             MARGIN + 5 * GW + MARGIN, TILE_W]
X_HEAD_SPLIT = 458    # x0 chunk0 sub-split: [0:458) covers taps 0-2
# images 1-3: two chunks
XA = MARGIN + 4 * GW + MARGIN  # 1940
N_BF_TAPS = 7         # taps 0..6 in bf16; taps 7,8 fused fp8 DoubleRow

# (image, row0, nrows) for every PSUM group; the last image ends with
# 6-row and 2-row groups to shorten the tail.
GROUPS = []
for _n in range(N_PER_CORE):
    if _n < N_PER_CORE - 1:
        GROUPS += [(_n, r, 8) for r in range(0, H, 8)]
    else:
        GROUPS += [(_n, r, 8) for r in range(0, 48, 8)]
        GROUPS += [(_n, 48, 6), (_n, 54, 2)]

_CACHE = {}


def _build_nc():
    import concourse.mybir as mybir
    import concourse.tile as tile
    from concourse import bacc
    from concourse.tile import add_dep_helper

    f32 = mybir.dt.float32
    bf16 = mybir.dt.bfloat16
    fp8 = mybir.dt.float8e4
    af = mybir.ActivationFunctionType
    DR = mybir.MatmulPerfMode.DoubleRow

    nc = bacc.Bacc("TRN2", target_bir_lowering=False, debug=False)

    x_d = nc.dram_tensor("x", [N_PER_CORE, C, TILE_W], bf16, kind="ExternalInput")
    x8_d = nc.dram_tensor("x8", [N_PER_CORE, C, 2 * TILE_W], fp8,
                          kind="ExternalInput")
    w_d = nc.dram_tensor("w", [C, N_BF_TAPS * C], bf16, kind="ExternalInput")
    w8_d = nc.dram_tensor("w8", [C, 2 * C], fp8, kind="ExternalInput")
    b_d = nc.dram_tensor("b", [C, 1], f32, kind="ExternalInput")
    y_d = nc.dram_tensor("y", [N_PER_CORE, C, HW], bf16, kind="ExternalOutput")

    with tile.TileContext(nc) as tc:
        with (
            tc.tile_pool(name="const", bufs=1) as cpool,
            tc.tile_pool(name="xbuf", bufs=1) as xpool,
            tc.tile_pool(name="obuf", bufs=2) as opool,
            tc.tile_pool(name="psum", bufs=8, space="PSUM") as ppool,
        ):
            # PE warm-up on a zero scratch (HAM clock ramp), started as
            # early as possible: gpsimd does the (small) memset because
            # its instruction stream starts earliest.
            zsc = cpool.tile([C, WARM_N], bf16, tag="zsc")
            nc.gpsimd.memset(zsc[:], 0.0)
            for _ in range(N_WARM):
                wm = ppool.tile([C, WARM_N], f32, tag="ps", name="ps")
                nc.tensor.matmul(wm[:], zsc[:, 0:C], zsc[:], start=True, stop=True)

            xts = [xpool.tile([C, TILE_W], bf16, tag=f"x{n}", name=f"x{n}")
                   for n in range(N_PER_CORE)]
            x8ts = [xpool.tile([C, 2, TILE_W], fp8, tag=f"x8_{n}", name=f"x8_{n}")
                    for n in range(N_PER_CORE)]
            wt = cpool.tile([C, N_BF_TAPS * C], bf16, tag="wt")
            w8t = cpool.tile([C, 2, C], fp8, tag="w8t")
            bt = cpool.tile([C, 1], f32, tag="bt")

            # critical head transfers (only sync + scalar have HW DGE):
            # sync carries the bf16 weights in tap order, chunk sizes
            # matched to the cold-clock 380ns tap pitch; scalar carries
            # x0's bf16 chunk0 (the true gate on the first conv matmul).
            nc.sync.dma_start(out=wt[:, 0:C], in_=w_d[:, 0:C])
            nc.scalar.dma_start(
                out=xts[0][:, 0:X_HEAD_SPLIT], in_=x_d[0][:, 0:X_HEAD_SPLIT]
            )
            for ta, tb in ((1, 3), (3, 5), (5, 7)):
                nc.sync.dma_start(
                    out=wt[:, ta * C : tb * C], in_=w_d[:, ta * C : tb * C]
                )
            nc.sync.dma_start(out=w8t[:], in_=w8_d[:])
            nc.sync.dma_start(out=bt[:], in_=b_d[:])
            # x0 bf16 chunk1 rides on sync (idle after the weights);
            # scalar finishes x0 chunk0 then brings the fp8 chunk0
            nc.scalar.dma_start(
                out=xts[0][:, X_HEAD_SPLIT : X_BOUNDS0[1]],
                in_=x_d[0][:, X_HEAD_SPLIT : X_BOUNDS0[1]],
            )
            nc.scalar.dma_start(
                out=x8ts[0][:, :, 0 : X_BOUNDS0[1]],
                in_=x8_d[0].rearrange("p (two w) -> p two w", two=2)[
                    :, :, 0 : X_BOUNDS0[1]
                ],
            )
            nc.sync.dma_start(
                out=xts[0][:, X_BOUNDS0[1] : X_BOUNDS0[2]],
                in_=x_d[0][:, X_BOUNDS0[1] : X_BOUNDS0[2]],
            )
            x_dmas = {0: [], 1: [], 2: [], 3: []}
            # tails of image 0 (gated on compute below): the fp8 planes
            # chunked on scalar (group g's DR matmul must not wait on a
            # whole-image transfer), the bf16 tail on sync (idle after
            # the weight chunks until the output stream starts)
            x8v0 = x8_d[0].rearrange("p (two w) -> p two w", two=2)
            for a, b in zip(X_BOUNDS0[1:], X_BOUNDS0[2:]):
                x_dmas[0].append(nc.scalar.dma_start(
                    out=x8ts[0][:, :, a:b], in_=x8v0[:, :, a:b]))
            for a, b in zip(X_BOUNDS0[2:], X_BOUNDS0[3:]):
                x_dmas[0].append(
                    nc.sync.dma_start(out=xts[0][:, a:b], in_=x_d[0][:, a:b])
                )
            # images 1-3 (gated on compute below): fp8 chunks on scalar;
            # bf16 on sync except image 3's (scalar, to keep sync's queue
            # short near the tail)
            for n in range(1, N_PER_CORE):
                x8v = x8_d[n].rearrange("p (two w) -> p two w", two=2)
                x_dmas[n].append(nc.scalar.dma_start(
                    out=x8ts[n][:, :, 0:XA], in_=x8v[:, :, 0:XA]))
                x_dmas[n].append(nc.scalar.dma_start(
                    out=x8ts[n][:, :, XA:TILE_W], in_=x8v[:, :, XA:TILE_W]))
                eng = nc.scalar if n == N_PER_CORE - 1 else nc.sync
                x_dmas[n].append(eng.dma_start(
                    out=xts[n][:, 0:XA], in_=x_d[n][:, 0:XA]))
                x_dmas[n].append(eng.dma_start(
                    out=xts[n][:, XA:TILE_W], in_=x_d[n][:, XA:TILE_W]))

            ots = {}
            gate0_mms = {}  # image -> first matmul of its group 0
            gate_mms = {}   # image -> last matmul of its group 0
            for n, row0, nrows in GROUPS:
                if n not in ots:
                    ots[n] = opool.tile([C, HW], bf16, tag="ot", name=f"ot{n}")
                ot = ots[n]
                width = nrows * WP
                ps = ppool.tile([C, GW], f32, tag="ps", name="ps")
                for t in range(N_BF_TAPS):
                    kh, kw = divmod(t, 3)
                    d = (kh - 1) * WP + (kw - 1)
                    base = MARGIN + row0 * WP + d
                    mm = nc.tensor.matmul(
                        ps[:, 0:width], wt[:, t * C : (t + 1) * C],
                        xts[n][:, base : base + width],
                        start=(t == 0), stop=False,
                    )
                    if row0 == 0 and t == 0:
                        gate0_mms[n] = mm
                # taps 7 (kh=2,kw=1,d=+57) and 8 (kh=2,kw=2,d=+58) as one
                # fp8 DoubleRow matmul: plane0 = x8, plane1 = x8 shifted
                # by one column, so plane j gives tap 7+j's window
                base = MARGIN + row0 * WP + WP
                mm = nc.tensor.matmul(
                    ps[:, 0:width], w8t[:, 0:2, 0:C],
                    x8ts[n][:, 0:2, base : base + width],
                    start=False, stop=True, perf_mode=DR,
                )
                if row0 == 0:
                    gate_mms[n] = mm
                ni = nrows * W
                src = ps[:, 0:width]
                src = src.rearrange("p (r c) -> p r c", c=WP)[:, :, 0:W]
                dstp = ot[:, row0 * W : row0 * W + ni]
                dstp = dstp.rearrange("p (r c) -> p r c", c=W)
                if nrows == 2:
                    # final tiny group: evict on DVE so it doesn't wait
                    # for the scalar engine to finish the previous group;
                    # sync (free — the 6-row DMA went to scalar) triggers
                    # the final DMA
                    nc.vector.tensor_scalar_add(dstp, src, bt[:])
                    nc.sync.dma_start(
                        out=y_d[n][:, row0 * W : row0 * W + ni],
                        in_=ot[:, row0 * W : row0 * W + ni],
                    )
                else:
                    nc.scalar.activation(dstp, src, af.Identity, bias=bt[:])
                    if nrows == 6:
                        # second-to-last group: scalar triggers its own DMA
                        nc.scalar.dma_start(
                            out=y_d[n][:, row0 * W : row0 * W + ni],
                            in_=ot[:, row0 * W : row0 * W + ni],
                        )
                    else:
                        nc.sync.dma_start(
                            out=y_d[n][:, row0 * W : row0 * W + ni],
                            in_=ot[:, row0 * W : row0 * W + ni],
                        )

            # stagger the bulk input transfers behind compute milestones
            # so they don't contend with the critical head transfers; the
            # gates fire EARLY (first tap of the previous image's first
            # group) because the gate->trigger->DGE->transfer->semaphore
            # chain is ~3us and must land before the consumer group
            for dma in x_dmas[0]:
                add_dep_helper(
                    dma.ins, gate0_mms[0].ins, sync=True,
                    reason="stagger x0 tail",
                )
            for n in range(1, N_PER_CORE):
                for dma in x_dmas[n]:
                    add_dep_helper(
                        dma.ins, gate_mms[n - 1].ins, sync=True,
                        reason="stagger x prefetch",
                    )

    nc.compile()
    return nc


def _get_nc():
    if "nc" not in _CACHE:
        _CACHE["nc"] = _build_nc()
    return _CACHE["nc"]


def _prep_inputs(x, weights, bias):
    import ml_dtypes

    bf16 = ml_dtypes.bfloat16
    e4m3 = ml_dtypes.float8_e4m3
    x = np.asarray(x, dtype=np.float32).reshape(N_TOTAL, C, H, W)
    # bf16 flat padded tiles
    xp = np.zeros((N_TOTAL, C, TILE_W), dtype=bf16)
    v = xp[:, :, MARGIN : MARGIN + L].reshape(N_TOTAL, C, H, WP)
    v[:, :, :, 0:W] = x.astype(bf16)
    # fp8 two-plane tiles: plane0 = e4m3(x) flat padded, plane1 shifted
    # left by one column (so DR plane j = tap 7+j's window)
    x8flat = np.zeros((N_TOTAL, C, TILE_W), dtype=e4m3)
    v8 = x8flat[:, :, MARGIN : MARGIN + L].reshape(N_TOTAL, C, H, WP)
    v8[:, :, :, 0:W] = x.astype(e4m3)
    x8p = np.zeros((N_TOTAL, C, 2, TILE_W), dtype=e4m3)
    x8p[:, :, 0, :] = x8flat
    x8p[:, :, 1, :-1] = x8flat[:, :, 1:]
    x8p = np.ascontiguousarray(x8p.reshape(N_TOTAL, C, 2 * TILE_W))
    # weights [co, ci, kh, kw] -> [ci, tap, co]; taps 0-6 bf16, taps 7,8
    # as fp8 pair [ci, 2, co]
    w = np.asarray(weights, dtype=np.float32)
    wT = np.transpose(w, (1, 2, 3, 0)).reshape(C, 9, C)
    wbf = np.ascontiguousarray(wT[:, 0:N_BF_TAPS, :].reshape(C, N_BF_TAPS * C)
                               ).astype(bf16)
    w8 = np.ascontiguousarray(wT[:, N_BF_TAPS:9, :].reshape(C, 2 * C)
                              ).astype(e4m3)
    b = np.ascontiguousarray(np.asarray(bias, dtype=np.float32).reshape(C, 1))
    return xp, x8p, wbf, w8, b


def kernel(x, weights, bias, _trace=False):
    from concourse.bass_utils import run_bass_kernel_spmd

    nc = _get_nc()
    xp, x8p, wbf, w8, b = _prep_inputs(x, weights, bias)
    in_maps = [
        {
            "x": xp[i * N_PER_CORE : (i + 1) * N_PER_CORE],
            "x8": x8p[i * N_PER_CORE : (i + 1) * N_PER_CORE],
            "w": wbf, "w8": w8, "b": b,
        }
        for i in range(N_CORES)
    ]
    res = run_bass_kernel_spmd(
        nc, in_maps, core_ids=list(range(N_CORES)), trace=_trace
    )
    y = np.concatenate([r["y"] for r in res.results], axis=0)
    y = y.astype(np.float32).reshape(N_TOTAL, C, H, W)
    if _trace:
        return y, res
    return y
